# revision 10
# baseline (speedup 1.0000x reference)
# Contextual loss kernel for Trainium2, 8 NeuronCores.
#
# Reference computation:
#   y_mu = mean(y, axis=(0,2,3))                       # per channel
#   xn = normalize(x - y_mu, axis=C); yn = normalize(y - y_mu, axis=C)
#   A[n,p,q] = sum_c xn[n,c,p] * yn[n,c,q]             # cosine similarity
#   dist = 1 - A;  dist_tilde = dist / (min_q dist + EPS)
#   w = exp((1 - dist_tilde)/bw);  cx = w / sum_q w
#   loss = mean_n(-log(mean_q max_p cx + EPS))
#
# Exponent algebra: (1 - dist_tilde)/bw = t*A + b with
#   t = 1/(bw*(1 + EPS - rmax)),  b = 1/bw - t,  rmax = max_q A  (per row).
#
# Sharding: core c handles sample n=c//2, row-half h=c%2 (2048 of the 4096
# p-rows). Each core returns the per-column partial max m_q of cx over its
# rows; the host combines halves (elementwise max), means over q, -log/mean.
#
# Single-pass main loop per 128-row block (vs. the old two-pass design that
# recomputed every matmul):
#   PE  : A-half [128,2048] into PSUM (two PSUM buffers rotate)
#   DVE : row-max of each PSUM half (tensor_reduce)
#   ACT : evacuate PSUM -> SBUF fp16 A, folding the row scale 1/||xc_p||
#         via the activation Copy per-partition scale operand
#   DVE : tiny [128,1] chain -> exp scale t and bias b
#   ACT : w = Exp(t*A + b) from SBUF fp16, fused row-sum accumulator S
#   DVE : v = w * (1/S)            (tensor_scalar, 4x mode on bf16)
#   DVE : Macc = max(Macc, v)      (tensor_tensor, 2x mode on bf16)
# (tensor_tensor_reduce / custom-DVE ISA ops would fuse evac+max into one
# DVE pass, but every InstISA op dies in this walrus build's codegen with
# "ISA wrong length", so only plain BIR instructions are used.)
# The v/TT tail of block r is emitted one/two iterations later so the
# in-order DVE queue never stalls waiting on ACT.
# Final fold of Macc [128,4096] across partitions via PE transpose + DVE
# reduce_max -> m[4096].

import numpy as np

N, C, H, W = 4, 256, 64, 64
P = H * W            # 4096
HALF = P // 2        # 2048
NBLK = HALF // 128   # 16
NCORES = 8
BW = 0.5
EPS = 1e-5
NEG_INIT = -1.0e30

FP8 = True          # main matmul in fp8e4m3 DoubleRow (2x PE) vs bf16
WARMUP_CC = False     # issue a dummy AllReduce at t=0 to absorb CC setup

_cache = {}


def _patched_tile_context(tile_mod, nc):
    """TileContext whose tail drain splits its sem waits one-per-drain.

    The walrus build in this container rejects a Drain instruction carrying
    more than one sync wait ("Too many sync wait commands"), and the stock
    TileContext attaches the whole global clock to a single drain.
    """
    from concourse.vector_clock import ScopedClock

    class TC(tile_mod.TileContext):
        def _drain_and_barrier(self, tick_clock, wait_clock):
            nc_ = self.nc
            drain_inst = nc_.sync.drain()
            wait_clock.add_sem_waits(
                drain_inst.ins, ScopedClock({None: tick_clock.global_clock})
            )
            si = drain_inst.ins.sync_info
            waits = list(si.on_wait or []) if si is not None else []
            if len(waits) > 1:
                si.on_wait = waits[:1]
                rest = waits[1:]
                while rest:
                    d2 = nc_.sync.drain()
                    if d2.ins.sync_info is None:
                        d2.ins.sync_info = type(si)(on_wait=rest[:1], on_update=[])
                    else:
                        d2.ins.sync_info.on_wait = rest[:1]
                    rest = rest[1:]
            nc_.all_engine_barrier()
            assert self.sems is not None
            popped = nc_._tile_sem_poison_stack.pop()
            assert popped is self._sem_poison
            nc_.clear_and_free_semaphores(list(self.sems.allocated().values()))
            nc_.all_engine_barrier()

    return TC(nc)


def _split_excess_waits(nc, mybir, maxw=1):
    """Hoist sync waits beyond `maxw` per instruction onto EventSemaphore
    carrier instructions inserted just before, on the same engine."""
    k = 0
    for fn in nc.m.functions:
        for blk in fn.blocks:
            il = blk.instructions
            new = []
            changed = False
            for ins in il:
                si = getattr(ins, "sync_info", None)
                waits = list(si.on_wait) if (si is not None and si.on_wait) else []
                if len(waits) > maxw:
                    changed = True
                    extra, keep = waits[:-maxw], waits[-maxw:]
                    while extra:
                        chunk, extra = extra[:maxw], extra[maxw:]
                        ev = mybir.InstEventSemaphore(name=f"I-sw{k}")
                        k += 1
                        ev.engine = ins.engine
                        ev.sync_info = type(si)(on_wait=chunk, on_update=[])
                        new.append(ev)
                    si.on_wait = keep
                new.append(ins)
            if changed:
                blk.instructions = new


def _bcast(ap_col, n):
    """[128,1] column slice -> [128,n] stride-0 free-dim broadcast AP."""
    import concourse.bass as bass

    return bass.AP(
        tensor=ap_col.tensor, offset=ap_col.offset, ap=[ap_col.ap[0], [0, n]]
    )


def _inv_sqrt(nc, mybir, pool, nsq, out, tag):
    """out = 1/sqrt(nsq), ACT sqrt + DVE reciprocal + one Newton step."""
    OP = mybir.AluOpType
    AF = mybir.ActivationFunctionType
    shape = list(nsq.shape)
    t = pool.tile(shape, mybir.dt.float32, tag=f"isq_t{tag}", name=f"isq_t{tag}")
    nc.scalar.activation(out=t, in_=nsq, func=AF.Sqrt)
    r = pool.tile(shape, mybir.dt.float32, tag=f"isq_r{tag}", name=f"isq_r{tag}")
    nc.vector.reciprocal(r, t)
    e = pool.tile(shape, mybir.dt.float32, tag=f"isq_e{tag}", name=f"isq_e{tag}")
    nc.vector.tensor_mul(e, r, r)
    nc.vector.tensor_mul(e, e, nsq)
    nc.vector.tensor_scalar(
        out=e, in0=e, scalar1=-0.5, scalar2=1.5, op0=OP.mult, op1=OP.add
    )
    nc.vector.tensor_mul(out, r, e)


def _build_nc():
    from contextlib import ExitStack

    import concourse.bass as bass
    import concourse.tile as tile
    from concourse import mybir
    from concourse.masks import make_identity

    fp32 = mybir.dt.float32
    fp16 = mybir.dt.float16
    bf16 = mybir.dt.bfloat16
    X = mybir.AxisListType.X
    OP = mybir.AluOpType
    AF = mybir.ActivationFunctionType

    mm_dt = mybir.dt.float8e4 if FP8 else bf16
    # fp8 inputs: y-side scaled by S8 (unit-norm columns ~1/16 per entry);
    # the x side stays ~N(0,1). 1/S8 is folded into the inx chain.
    S8 = 16.0 if FP8 else 1.0

    nc = bass.Bass("TRN2", target_bir_lowering=False)
    xh_d = nc.declare_dram_parameter("xh", [C, HALF], fp32, isOutput=False)
    yn_d = nc.declare_dram_parameter("yn", [C, P], fp32, isOutput=False)
    m_d = nc.declare_dram_parameter("m_out", [32, 128], fp32, isOutput=True)

    with _patched_tile_context(tile, nc) as tc, ExitStack() as ctx:
        const = ctx.enter_context(tc.tile_pool(name="const", bufs=1))
        persist = ctx.enter_context(tc.tile_pool(name="persist", bufs=1))
        dram = ctx.enter_context(tc.tile_pool(name="dram", bufs=1, space="DRAM"))

        ones_b = const.tile([128, 1], bf16)
        nc.vector.memset(ones_b, 1.0)
        ident = const.tile([128, 128], bf16)
        make_identity(nc, ident)
        zero2 = const.tile([128, 2], fp32)
        nc.vector.memset(zero2, 0.0)

        # persistent tiles
        ynb = persist.tile([128, 2, P], mm_dt, tag="ynb")
        xnb = persist.tile([128, 2, HALF], mm_dt, tag="xnb")
        inx = persist.tile([128, NBLK], fp32, tag="inx")
        Macc = persist.tile([128, P], bf16, tag="Macc")
        mfold = persist.tile([128, 32], fp32, tag="mfold")
        negmu = persist.tile([128, 2], fp32, tag="negmu")
        # per-block [128,1] scalars as columns
        rm8 = persist.tile([128, 8], fp16, tag="rm8")
        bwd = persist.tile([128, NBLK], fp32, tag="bwd")
        tsc = persist.tile([128, NBLK], fp32, tag="tsc")
        bsc = persist.tile([128, NBLK], fp32, tag="bsc")
        SS = persist.tile([128, NBLK], fp32, tag="SS")
        iS = persist.tile([128, NBLK], fp32, tag="iS")

        nc.vector.memset(Macc, 0.0)

        # ---------------- phase 0: warmup CC + loads + y_mu AllReduce -------
        if WARMUP_CC:
            warm_in = dram.tile([128, 2], fp32, name="warm_in")
            warm_out = dram.tile([128, 2], fp32, name="warm_out")
            nc.sync.dma_start(out=warm_in[:, :], in_=zero2)
            nc.gpsimd.collective_compute(
                "AllReduce",
                OP.add,
                replica_groups=[list(range(NCORES))],
                ins=[warm_in[:, :]],
                outs=[warm_out[:, :]],
            )

        prep = ctx.enter_context(tc.tile_pool(name="prep", bufs=1))
        yc2 = prep.tile([128, 2, P], fp32, tag="yc2", name="yc2")
        xc2 = prep.tile([128, 2, HALF], fp32, tag="xc2", name="xc2")
        nc.sync.dma_start(out=yc2, in_=yn_d[:, :].rearrange("(a c) q -> c a q", a=2))
        nc.sync.dma_start(out=xc2, in_=xh_d[:, :].rearrange("(a c) q -> c a q", a=2))

        part2 = prep.tile([128, 2], fp32, tag="part2")
        for h in range(2):
            nc.vector.tensor_reduce(
                out=part2[:, h : h + 1], in_=yc2[:, h : h + 1, :], axis=X, op=OP.add
            )
        cc_in = dram.tile([128, 2], fp32, name="cc_in")
        cc_out = dram.tile([128, 2], fp32, name="cc_out")
        nc.sync.dma_start(out=cc_in[:, :], in_=part2)
        nc.gpsimd.collective_compute(
            "AllReduce",
            OP.add,
            replica_groups=[list(range(NCORES))],
            ins=[cc_in[:, :]],
            outs=[cc_out[:, :]],
        )
        allred = prep.tile([128, 2], fp32, tag="allred")
        nc.sync.dma_start(out=allred, in_=cc_out[:, :])
        nc.vector.tensor_scalar_mul(
            out=negmu, in0=allred, scalar1=-1.0 / float(2 * N * P)
        )

        # ---------------- phase 1: center, norms, casts ---------------------
        with tc.tile_pool(name="ph1ps", bufs=1, space="PSUM") as ph1ps:
            nrm_ps = ph1ps.tile([1, P], fp32, tag="nrm")

            # y side: center, square(bf16), column norms via ones-matmul
            ysq = prep.tile([128, 2, P], bf16, tag="ysq", name="ysq")
            for h in range(2):
                nc.vector.tensor_scalar_add(
                    out=yc2[:, h : h + 1, :],
                    in0=yc2[:, h : h + 1, :],
                    scalar1=negmu[:, h : h + 1],
                )
                nc.scalar.activation(
                    out=ysq[:, h : h + 1, :], in_=yc2[:, h : h + 1, :], func=AF.Square
                )
            for h in range(2):
                for j in range(P // 512):
                    nc.tensor.matmul(
                        nrm_ps[0:1, j * 512 : (j + 1) * 512],
                        lhsT=ones_b,
                        rhs=ysq[:, h, j * 512 : (j + 1) * 512],
                        start=(h == 0),
                        stop=(h == 1),
                    )
            nrm_sb = prep.tile([1, P], fp32, tag="nrm_sb")
            nc.scalar.copy(nrm_sb, nrm_ps[0:1, :])
            dy = dram.tile([32, 128], fp32, tag="dy")
            nc.sync.dma_start(
                out=dy[:, :].rearrange("j p -> (j p)").rearrange("(a f) -> a f", a=1),
                in_=nrm_sb[0:1, :],
            )
            nsq_y = prep.tile([128, 32], fp32, tag="nsq_y")
            nc.sync.dma_start(out=nsq_y, in_=dy[:, :].rearrange("j p -> p j"))

            # x side: center, square(bf16), norms; cast xnb
            xsq = prep.tile([128, 2, HALF], bf16, tag="xsq", name="xsq")
            for h in range(2):
                nc.vector.tensor_scalar_add(
                    out=xc2[:, h : h + 1, :],
                    in0=xc2[:, h : h + 1, :],
                    scalar1=negmu[:, h : h + 1],
                )
                nc.scalar.activation(
                    out=xsq[:, h : h + 1, :], in_=xc2[:, h : h + 1, :], func=AF.Square
                )
            nc.vector.tensor_copy(xnb, xc2)
            for h in range(2):
                for j in range(HALF // 512):
                    nc.tensor.matmul(
                        nrm_ps[0:1, j * 512 : (j + 1) * 512],
                        lhsT=ones_b,
                        rhs=xsq[:, h, j * 512 : (j + 1) * 512],
                        start=(h == 0),
                        stop=(h == 1),
                    )
            nrmx_sb = prep.tile([1, HALF], fp32, tag="nrmx_sb")
            nc.scalar.copy(nrmx_sb, nrm_ps[0:1, 0:HALF])
            dx = dram.tile([16, 128], fp32, tag="dx")
            nc.sync.dma_start(
                out=dx[:, :].rearrange("j p -> (j p)").rearrange("(a f) -> a f", a=1),
                in_=nrmx_sb[0:1, :],
            )
            nsq_x = prep.tile([128, NBLK], fp32, tag="nsq_x")
            nc.sync.dma_start(out=nsq_x, in_=dx[:, :].rearrange("j p -> p j"))

            # inverse norms; y goes back out through DRAM as a broadcast
            iny = prep.tile([128, 32], fp32, tag="iny")
            _inv_sqrt(nc, mybir, prep, nsq_y, iny, tag="y")
            if S8 != 1.0:
                nc.vector.tensor_scalar_mul(out=iny, in0=iny, scalar1=S8)
            dyb = dram.tile([32, 128], fp32, tag="dyb")
            nc.sync.dma_start(out=dyb[:, :].rearrange("j p -> p j"), in_=iny)
            inyb = prep.tile([128, P], fp32, tag="inyb")
            src = bass.AP(tensor=dyb.tensor, offset=dyb.offset, ap=[[0, 128], [1, P]])
            nc.sync.dma_start(out=inyb, in_=src)

            inx_pre = prep.tile([128, NBLK], fp32, tag="inx_pre")
            _inv_sqrt(nc, mybir, prep, nsq_x, inx_pre, tag="x")
            nc.vector.tensor_scalar_mul(out=inx, in0=inx_pre, scalar1=1.0 / S8)

            # normalized y in matmul dtype: ynb = yc * iny (column scale)
            in1 = bass.AP(
                tensor=inyb.tensor,
                offset=inyb.offset,
                ap=[inyb.ap[0], [0, 2], [1, P]],
            )
            nc.vector.tensor_tensor(out=ynb, in0=yc2, in1=in1, op=OP.mult)

        # ---------------- phase 2: single-pass main loop --------------------
        with tc.tile_pool(name="mmps", bufs=2, space="PSUM") as mmps, tc.tile_pool(
            name="apool", bufs=2
        ) as apool, tc.tile_pool(name="wpool", bufs=3) as wpool, tc.tile_pool(
            name="vpool", bufs=3
        ) as vpool:
            # stage queues for the depth-3 software pipeline:
            #   iter r emits mm/evac/max8/chain(r), exp(r-1), iS+v(r-2), TT(r-3)
            st_exp = []  # r with A_/chain ready, exp not yet emitted
            st_va = []   # (r, w_) awaiting iS + v
            st_tt = []   # v_ awaiting the Macc TT max

            def emit_exp(r, A_):
                w_ = wpool.tile([128, P], bf16, tag="w", name=f"w{r}")
                nc.scalar.activation(
                    out=w_,
                    in_=A_,
                    func=AF.Exp,
                    bias=bsc[:, r : r + 1],
                    scale=tsc[:, r : r + 1],
                    accum_out=SS[:, r : r + 1],
                )
                return w_

            def emit_va(r, w_):
                nc.vector.reciprocal(iS[:, r : r + 1], SS[:, r : r + 1])
                v_ = vpool.tile([128, P], bf16, tag="v", name=f"v{r}")
                nc.vector.tensor_scalar_mul(out=v_, in0=w_, scalar1=iS[:, r : r + 1])
                return v_

            def emit_tt(v_):
                nc.vector.tensor_tensor(out=Macc, in0=Macc, in1=v_, op=OP.max)

            def pump(drain=False):
                # advance each stage at most one block per call
                if st_tt:
                    emit_tt(st_tt.pop(0))
                if st_va:
                    pr, pw = st_va.pop(0)
                    st_tt.append(emit_va(pr, pw))
                if st_exp:
                    pr, pA = st_exp.pop(0)
                    st_va.append((pr, emit_exp(pr, pA)))

            for r in range(NBLK):
                A_ = apool.tile([128, P], fp16, tag="A", name=f"A{r}")
                for half in range(2):
                    ps = mmps.tile([128, HALF], fp32, tag="ps", name=f"ps{r}_{half}")
                    lhsT = xnb[:, :, r * 128 : (r + 1) * 128]
                    for j in range(HALF // 512):
                        q0 = half * HALF + j * 512
                        if FP8:
                            nc.tensor.matmul(
                                ps[:, j * 512 : (j + 1) * 512],
                                lhsT=lhsT,
                                rhs=ynb[:, :, q0 : q0 + 512],
                                perf_mode=mybir.MatmulPerfMode.DoubleRow,
                            )
                        else:
                            for h in range(2):
                                nc.tensor.matmul(
                                    ps[:, j * 512 : (j + 1) * 512],
                                    lhsT=xnb[:, h, r * 128 : (r + 1) * 128],
                                    rhs=ynb[:, h, q0 : q0 + 512],
                                    start=(h == 0),
                                    stop=(h == 1),
                                )
                    nc.scalar.mul(
                        A_[:, half * HALF : (half + 1) * HALF],
                        ps,
                        inx[:, r : r + 1],
                    )
                # row max from the normalized fp16 A via Max8 (top-8/row)
                nc.vector.max(out=rm8, in_=A_)
                # chain: t = 1/(bw*(1+eps-rmax)); b = 1/bw - t
                nc.vector.tensor_scalar(
                    out=bwd[:, r : r + 1],
                    in0=rm8[:, 0:1],
                    scalar1=-BW,
                    scalar2=BW * (1.0 + EPS),
                    op0=OP.mult,
                    op1=OP.add,
                )
                nc.vector.reciprocal(tsc[:, r : r + 1], bwd[:, r : r + 1])
                nc.vector.tensor_scalar(
                    out=bsc[:, r : r + 1],
                    in0=tsc[:, r : r + 1],
                    scalar1=-1.0,
                    scalar2=1.0 / BW,
                    op0=OP.mult,
                    op1=OP.add,
                )
                st_exp.append((r, A_))
                pump()
            while st_exp or st_va or st_tt:
                pump(drain=True)

        # ---------------- phase 3: fold M across partitions -----------------
        with tc.tile_pool(name="tps", bufs=4, space="PSUM") as tps:
            for j in range(P // 128):
                pt = tps.tile([128, 128], bf16, tag="pt")
                nc.tensor.transpose(pt, Macc[:, j * 128 : (j + 1) * 128], ident)
                nc.vector.tensor_reduce(
                    out=mfold[:, j : j + 1], in_=pt, axis=X, op=OP.max
                )
        nc.sync.dma_start(out=m_d[:, :].rearrange("j p -> p j"), in_=mfold)

    _split_excess_waits(nc, mybir, maxw=1)
    return nc


def kernel(x, y):
    from concourse.bass_utils import run_bass_kernel_spmd

    x = np.ascontiguousarray(np.asarray(x, dtype=np.float32))
    y = np.ascontiguousarray(np.asarray(y, dtype=np.float32))
    assert x.shape == (N, C, H, W) and y.shape == (N, C, H, W)

    if "nc" not in _cache:
        _cache["nc"] = _build_nc()
    nc = _cache["nc"]

    in_maps = []
    for c in range(NCORES):
        n, h = c // 2, c % 2
        in_maps.append(
            {
                "xh": np.ascontiguousarray(
                    x[n].reshape(C, P)[:, h * HALF : (h + 1) * HALF]
                ),
                "yn": np.ascontiguousarray(y[n].reshape(C, P)),
            }
        )
    res = run_bass_kernel_spmd(nc, in_maps, core_ids=list(range(NCORES)))
    ms = [r["m_out"].reshape(P) for r in res.results]
    cx = np.empty(N, np.float64)
    for n in range(N):
        m = np.maximum(ms[2 * n], ms[2 * n + 1])
        cx[n] = m.astype(np.float64).mean()
    loss = np.mean(-np.log(cx + EPS))
    return np.asarray(loss, dtype=np.float32)


# revision 11
# speedup vs baseline: 1.0039x; 1.0039x over previous
# Contextual loss kernel for Trainium2, 8 NeuronCores.
#
# Reference computation:
#   y_mu = mean(y, axis=(0,2,3))                       # per channel
#   xn = normalize(x - y_mu, axis=C); yn = normalize(y - y_mu, axis=C)
#   A[n,p,q] = sum_c xn[n,c,p] * yn[n,c,q]             # cosine similarity
#   dist = 1 - A;  dist_tilde = dist / (min_q dist + EPS)
#   w = exp((1 - dist_tilde)/bw);  cx = w / sum_q w
#   loss = mean_n(-log(mean_q max_p cx + EPS))
#
# Exponent algebra: (1 - dist_tilde)/bw = t*A + b with
#   t = 1/(bw*(1 + EPS - rmax)),  b = 1/bw - t,  rmax = max_q A  (per row).
#
# Sharding: core c handles sample n=c//2, row-half h=c%2 (2048 of the 4096
# p-rows). Each core returns the per-column partial max m_q of cx over its
# rows; the host combines halves (elementwise max), means over q, -log/mean.
#
# Single-pass main loop per 128-row block (vs. the old two-pass design that
# recomputed every matmul):
#   PE  : A-half [128,2048] into PSUM (two PSUM buffers rotate)
#   DVE : row-max of each PSUM half (tensor_reduce)
#   ACT : evacuate PSUM -> SBUF fp16 A, folding the row scale 1/||xc_p||
#         via the activation Copy per-partition scale operand
#   DVE : tiny [128,1] chain -> exp scale t and bias b
#   ACT : w = Exp(t*A + b) from SBUF fp16, fused row-sum accumulator S
#   DVE : v = w * (1/S)            (tensor_scalar, 4x mode on bf16)
#   DVE : Macc = max(Macc, v)      (tensor_tensor, 2x mode on bf16)
# (tensor_tensor_reduce / custom-DVE ISA ops would fuse evac+max into one
# DVE pass, but every InstISA op dies in this walrus build's codegen with
# "ISA wrong length", so only plain BIR instructions are used.)
# The v/TT tail of block r is emitted one/two iterations later so the
# in-order DVE queue never stalls waiting on ACT.
# Final fold of Macc [128,4096] across partitions via PE transpose + DVE
# reduce_max -> m[4096].

import numpy as np

N, C, H, W = 4, 256, 64, 64
P = H * W            # 4096
HALF = P // 2        # 2048
NBLK = HALF // 128   # 16
NCORES = 8
BW = 0.5
EPS = 1e-5
NEG_INIT = -1.0e30

FP8 = True          # main matmul in fp8e4m3 DoubleRow (2x PE) vs bf16
WARMUP_CC = False     # issue a dummy AllReduce at t=0 to absorb CC setup

_cache = {}


def _patched_tile_context(tile_mod, nc):
    """TileContext whose tail drain splits its sem waits one-per-drain.

    The walrus build in this container rejects a Drain instruction carrying
    more than one sync wait ("Too many sync wait commands"), and the stock
    TileContext attaches the whole global clock to a single drain.
    """
    from concourse.vector_clock import ScopedClock

    class TC(tile_mod.TileContext):
        def _drain_and_barrier(self, tick_clock, wait_clock):
            nc_ = self.nc
            drain_inst = nc_.sync.drain()
            wait_clock.add_sem_waits(
                drain_inst.ins, ScopedClock({None: tick_clock.global_clock})
            )
            si = drain_inst.ins.sync_info
            waits = list(si.on_wait or []) if si is not None else []
            if len(waits) > 1:
                si.on_wait = waits[:1]
                rest = waits[1:]
                while rest:
                    d2 = nc_.sync.drain()
                    if d2.ins.sync_info is None:
                        d2.ins.sync_info = type(si)(on_wait=rest[:1], on_update=[])
                    else:
                        d2.ins.sync_info.on_wait = rest[:1]
                    rest = rest[1:]
            nc_.all_engine_barrier()
            assert self.sems is not None
            popped = nc_._tile_sem_poison_stack.pop()
            assert popped is self._sem_poison
            nc_.clear_and_free_semaphores(list(self.sems.allocated().values()))
            nc_.all_engine_barrier()

    return TC(nc)


def _split_excess_waits(nc, mybir, maxw=1):
    """Hoist sync waits beyond `maxw` per instruction onto EventSemaphore
    carrier instructions inserted just before, on the same engine."""
    k = 0
    for fn in nc.m.functions:
        for blk in fn.blocks:
            il = blk.instructions
            new = []
            changed = False
            for ins in il:
                si = getattr(ins, "sync_info", None)
                waits = list(si.on_wait) if (si is not None and si.on_wait) else []
                if len(waits) > maxw:
                    changed = True
                    extra, keep = waits[:-maxw], waits[-maxw:]
                    while extra:
                        chunk, extra = extra[:maxw], extra[maxw:]
                        ev = mybir.InstEventSemaphore(name=f"I-sw{k}")
                        k += 1
                        ev.engine = ins.engine
                        ev.sync_info = type(si)(on_wait=chunk, on_update=[])
                        new.append(ev)
                    si.on_wait = keep
                new.append(ins)
            if changed:
                blk.instructions = new


def _bcast(ap_col, n):
    """[128,1] column slice -> [128,n] stride-0 free-dim broadcast AP."""
    import concourse.bass as bass

    return bass.AP(
        tensor=ap_col.tensor, offset=ap_col.offset, ap=[ap_col.ap[0], [0, n]]
    )


def _inv_sqrt(nc, mybir, pool, nsq, out, tag):
    """out = 1/sqrt(nsq), ACT sqrt + DVE reciprocal + one Newton step."""
    OP = mybir.AluOpType
    AF = mybir.ActivationFunctionType
    shape = list(nsq.shape)
    t = pool.tile(shape, mybir.dt.float32, tag=f"isq_t{tag}", name=f"isq_t{tag}")
    nc.scalar.activation(out=t, in_=nsq, func=AF.Sqrt)
    r = pool.tile(shape, mybir.dt.float32, tag=f"isq_r{tag}", name=f"isq_r{tag}")
    nc.vector.reciprocal(r, t)
    e = pool.tile(shape, mybir.dt.float32, tag=f"isq_e{tag}", name=f"isq_e{tag}")
    nc.vector.tensor_mul(e, r, r)
    nc.vector.tensor_mul(e, e, nsq)
    nc.vector.tensor_scalar(
        out=e, in0=e, scalar1=-0.5, scalar2=1.5, op0=OP.mult, op1=OP.add
    )
    nc.vector.tensor_mul(out, r, e)


def _build_nc():
    from contextlib import ExitStack

    import concourse.bass as bass
    import concourse.tile as tile
    from concourse import mybir
    from concourse.masks import make_identity

    fp32 = mybir.dt.float32
    fp16 = mybir.dt.float16
    bf16 = mybir.dt.bfloat16
    X = mybir.AxisListType.X
    OP = mybir.AluOpType
    AF = mybir.ActivationFunctionType

    mm_dt = mybir.dt.float8e4 if FP8 else bf16
    # fp8 inputs: y-side scaled by S8 (unit-norm columns ~1/16 per entry);
    # the x side stays ~N(0,1). 1/S8 is folded into the inx chain.
    S8 = 16.0 if FP8 else 1.0

    nc = bass.Bass("TRN2", target_bir_lowering=False)
    xh_d = nc.declare_dram_parameter("xh", [C, HALF], fp32, isOutput=False)
    yn_d = nc.declare_dram_parameter("yn", [C, P], fp32, isOutput=False)
    m_d = nc.declare_dram_parameter("m_out", [32, 128], fp32, isOutput=True)

    with _patched_tile_context(tile, nc) as tc, ExitStack() as ctx:
        const = ctx.enter_context(tc.tile_pool(name="const", bufs=1))
        persist = ctx.enter_context(tc.tile_pool(name="persist", bufs=1))
        dram = ctx.enter_context(tc.tile_pool(name="dram", bufs=1, space="DRAM"))

        ones_b = const.tile([128, 1], bf16)
        nc.vector.memset(ones_b, 1.0)
        ident = const.tile([128, 128], bf16)
        make_identity(nc, ident)
        zero2 = const.tile([128, 2], fp32)
        nc.vector.memset(zero2, 0.0)
        ones_row = const.tile([1, 128], bf16)
        nc.vector.memset(ones_row, 1.0)

        # persistent tiles
        ynb = persist.tile([128, 2, P], mm_dt, tag="ynb")
        xnb = persist.tile([128, 2, HALF], mm_dt, tag="xnb")
        inx = persist.tile([128, NBLK], fp32, tag="inx")
        Macc = persist.tile([128, P], bf16, tag="Macc")
        mfold = persist.tile([128, 32], fp32, tag="mfold")
        negmu = persist.tile([128, 2], fp32, tag="negmu")
        # per-block [128,1] scalars as columns
        rm8 = persist.tile([128, 8], fp16, tag="rm8")
        bwd = persist.tile([128, NBLK], fp32, tag="bwd")
        tsc = persist.tile([128, NBLK], fp32, tag="tsc")
        bsc = persist.tile([128, NBLK], fp32, tag="bsc")
        SS = persist.tile([128, NBLK], fp32, tag="SS")
        iS = persist.tile([128, NBLK], fp32, tag="iS")

        nc.vector.memset(Macc, 0.0)

        # ---------------- phase 0: warmup CC + loads + y_mu AllReduce -------
        if WARMUP_CC:
            warm_in = dram.tile([128, 2], fp32, name="warm_in")
            warm_out = dram.tile([128, 2], fp32, name="warm_out")
            nc.sync.dma_start(out=warm_in[:, :], in_=zero2)
            nc.gpsimd.collective_compute(
                "AllReduce",
                OP.add,
                replica_groups=[list(range(NCORES))],
                ins=[warm_in[:, :]],
                outs=[warm_out[:, :]],
            )

        prep = ctx.enter_context(tc.tile_pool(name="prep", bufs=1))
        yc2 = prep.tile([128, 2, P], fp32, tag="yc2", name="yc2")
        xc2 = prep.tile([128, 2, HALF], fp32, tag="xc2", name="xc2")
        nc.sync.dma_start(out=yc2, in_=yn_d[:, :].rearrange("(a c) q -> c a q", a=2))
        nc.sync.dma_start(out=xc2, in_=xh_d[:, :].rearrange("(a c) q -> c a q", a=2))

        part2 = prep.tile([128, 2], fp32, tag="part2")
        for h in range(2):
            nc.vector.tensor_reduce(
                out=part2[:, h : h + 1], in_=yc2[:, h : h + 1, :], axis=X, op=OP.add
            )
        cc_in = dram.tile([128, 2], fp32, name="cc_in")
        cc_out = dram.tile([128, 2], fp32, name="cc_out")
        nc.sync.dma_start(out=cc_in[:, :], in_=part2)
        nc.gpsimd.collective_compute(
            "AllReduce",
            OP.add,
            replica_groups=[list(range(NCORES))],
            ins=[cc_in[:, :]],
            outs=[cc_out[:, :]],
        )
        allred = prep.tile([128, 2], fp32, tag="allred")
        nc.sync.dma_start(out=allred, in_=cc_out[:, :])
        nc.vector.tensor_scalar_mul(
            out=negmu, in0=allred, scalar1=-1.0 / float(2 * N * P)
        )

        # ---------------- phase 1: center, norms, casts ---------------------
        with tc.tile_pool(name="ph1ps", bufs=1, space="PSUM") as ph1ps:
            nrm_ps = ph1ps.tile([1, P], fp32, tag="nrm")

            # y side: center, square(bf16), column norms via ones-matmul
            ysq = prep.tile([128, 2, P], bf16, tag="ysq", name="ysq")
            for h in range(2):
                nc.vector.tensor_scalar_add(
                    out=yc2[:, h : h + 1, :],
                    in0=yc2[:, h : h + 1, :],
                    scalar1=negmu[:, h : h + 1],
                )
                nc.scalar.activation(
                    out=ysq[:, h : h + 1, :], in_=yc2[:, h : h + 1, :], func=AF.Square
                )
            for h in range(2):
                for j in range(P // 512):
                    nc.tensor.matmul(
                        nrm_ps[0:1, j * 512 : (j + 1) * 512],
                        lhsT=ones_b,
                        rhs=ysq[:, h, j * 512 : (j + 1) * 512],
                        start=(h == 0),
                        stop=(h == 1),
                    )
            nrm_sb = prep.tile([1, P], fp32, tag="nrm_sb")
            nc.scalar.copy(nrm_sb, nrm_ps[0:1, :])
            dy = dram.tile([32, 128], fp32, tag="dy")
            nc.sync.dma_start(
                out=dy[:, :].rearrange("j p -> (j p)").rearrange("(a f) -> a f", a=1),
                in_=nrm_sb[0:1, :],
            )
            nsq_y = prep.tile([128, 32], fp32, tag="nsq_y")
            nc.sync.dma_start(out=nsq_y, in_=dy[:, :].rearrange("j p -> p j"))

            # x side: center, square(bf16), norms; cast xnb
            xsq = prep.tile([128, 2, HALF], bf16, tag="xsq", name="xsq")
            for h in range(2):
                nc.vector.tensor_scalar_add(
                    out=xc2[:, h : h + 1, :],
                    in0=xc2[:, h : h + 1, :],
                    scalar1=negmu[:, h : h + 1],
                )
                nc.scalar.activation(
                    out=xsq[:, h : h + 1, :], in_=xc2[:, h : h + 1, :], func=AF.Square
                )
            nc.vector.tensor_copy(xnb, xc2)
            for h in range(2):
                for j in range(HALF // 512):
                    nc.tensor.matmul(
                        nrm_ps[0:1, j * 512 : (j + 1) * 512],
                        lhsT=ones_b,
                        rhs=xsq[:, h, j * 512 : (j + 1) * 512],
                        start=(h == 0),
                        stop=(h == 1),
                    )
            nrmx_sb = prep.tile([1, HALF], fp32, tag="nrmx_sb")
            nc.scalar.copy(nrmx_sb, nrm_ps[0:1, 0:HALF])
            dx = dram.tile([16, 128], fp32, tag="dx")
            nc.sync.dma_start(
                out=dx[:, :].rearrange("j p -> (j p)").rearrange("(a f) -> a f", a=1),
                in_=nrmx_sb[0:1, :],
            )
            nsq_x = prep.tile([128, NBLK], fp32, tag="nsq_x")
            nc.sync.dma_start(out=nsq_x, in_=dx[:, :].rearrange("j p -> p j"))

        # inverse norms; iny returns to a [1,P] row, then a K=1 ones-matmul
        # broadcasts it across partitions in PSUM (no 2MB DRAM broadcast DMA)
        iny = prep.tile([128, 32], fp32, tag="iny")
        _inv_sqrt(nc, mybir, prep, nsq_y, iny, tag="y")
        if S8 != 1.0:
            nc.vector.tensor_scalar_mul(out=iny, in0=iny, scalar1=S8)
        dyb = dram.tile([32, 128], fp32, tag="dyb")
        nc.sync.dma_start(out=dyb[:, :].rearrange("j p -> p j"), in_=iny)
        inyrow = prep.tile([1, P], fp32, tag="inyrow")
        nc.sync.dma_start(
            out=inyrow,
            in_=dyb[:, :].rearrange("j p -> (j p)").rearrange("(a f) -> a f", a=1),
        )
        inyrow16 = prep.tile([1, P], bf16, tag="inyrow16")
        nc.scalar.copy(inyrow16, inyrow[0:1, :])

        inx_pre = prep.tile([128, NBLK], fp32, tag="inx_pre")
        _inv_sqrt(nc, mybir, prep, nsq_x, inx_pre, tag="x")
        nc.vector.tensor_scalar_mul(out=inx, in0=inx_pre, scalar1=1.0 / S8)

        with tc.tile_pool(name="ph2ps", bufs=1, space="PSUM") as ph2ps:
            inyb_ps = ph2ps.tile([128, P], fp32, tag="inyb_ps")
            for j in range(P // 512):
                nc.tensor.matmul(
                    inyb_ps[:, j * 512 : (j + 1) * 512],
                    lhsT=ones_row,
                    rhs=inyrow16[0:1, j * 512 : (j + 1) * 512],
                )
            # normalized y in matmul dtype: ynb = yc * iny (column scale)
            in1 = bass.AP(
                tensor=inyb_ps.tensor,
                offset=inyb_ps.offset,
                ap=[inyb_ps.ap[0], [0, 2], [1, P]],
            )
            nc.vector.tensor_tensor(out=ynb, in0=yc2, in1=in1, op=OP.mult)

        # ---------------- phase 2: single-pass main loop --------------------
        with tc.tile_pool(name="mmps", bufs=2, space="PSUM") as mmps, tc.tile_pool(
            name="apool", bufs=2
        ) as apool, tc.tile_pool(name="wpool", bufs=3) as wpool, tc.tile_pool(
            name="vpool", bufs=3
        ) as vpool:
            # stage queues for the depth-3 software pipeline:
            #   iter r emits mm/evac/max8/chain(r), exp(r-1), iS+v(r-2), TT(r-3)
            st_exp = []  # r with A_/chain ready, exp not yet emitted
            st_va = []   # (r, w_) awaiting iS + v
            st_tt = []   # v_ awaiting the Macc TT max

            def emit_exp(r, A_):
                w_ = wpool.tile([128, P], bf16, tag="w", name=f"w{r}")
                nc.scalar.activation(
                    out=w_,
                    in_=A_,
                    func=AF.Exp,
                    bias=bsc[:, r : r + 1],
                    scale=tsc[:, r : r + 1],
                    accum_out=SS[:, r : r + 1],
                )
                return w_

            def emit_va(r, w_):
                nc.vector.reciprocal(iS[:, r : r + 1], SS[:, r : r + 1])
                v_ = vpool.tile([128, P], bf16, tag="v", name=f"v{r}")
                nc.vector.tensor_scalar_mul(out=v_, in0=w_, scalar1=iS[:, r : r + 1])
                return v_

            def emit_tt(v_):
                nc.vector.tensor_tensor(out=Macc, in0=Macc, in1=v_, op=OP.max)

            def pump(drain=False):
                # advance each stage at most one block per call
                if st_tt:
                    emit_tt(st_tt.pop(0))
                if st_va:
                    pr, pw = st_va.pop(0)
                    st_tt.append(emit_va(pr, pw))
                if st_exp:
                    pr, pA = st_exp.pop(0)
                    st_va.append((pr, emit_exp(pr, pA)))

            for r in range(NBLK):
                A_ = apool.tile([128, P], fp16, tag="A", name=f"A{r}")
                for half in range(2):
                    ps = mmps.tile([128, HALF], fp32, tag="ps", name=f"ps{r}_{half}")
                    lhsT = xnb[:, :, r * 128 : (r + 1) * 128]
                    for j in range(HALF // 512):
                        q0 = half * HALF + j * 512
                        if FP8:
                            nc.tensor.matmul(
                                ps[:, j * 512 : (j + 1) * 512],
                                lhsT=lhsT,
                                rhs=ynb[:, :, q0 : q0 + 512],
                                perf_mode=mybir.MatmulPerfMode.DoubleRow,
                            )
                        else:
                            for h in range(2):
                                nc.tensor.matmul(
                                    ps[:, j * 512 : (j + 1) * 512],
                                    lhsT=xnb[:, h, r * 128 : (r + 1) * 128],
                                    rhs=ynb[:, h, q0 : q0 + 512],
                                    start=(h == 0),
                                    stop=(h == 1),
                                )
                    nc.scalar.mul(
                        A_[:, half * HALF : (half + 1) * HALF],
                        ps,
                        inx[:, r : r + 1],
                    )
                # row max from the normalized fp16 A via Max8 (top-8/row)
                nc.vector.max(out=rm8, in_=A_)
                # chain: t = 1/(bw*(1+eps-rmax)); b = 1/bw - t
                nc.vector.tensor_scalar(
                    out=bwd[:, r : r + 1],
                    in0=rm8[:, 0:1],
                    scalar1=-BW,
                    scalar2=BW * (1.0 + EPS),
                    op0=OP.mult,
                    op1=OP.add,
                )
                nc.vector.reciprocal(tsc[:, r : r + 1], bwd[:, r : r + 1])
                nc.vector.tensor_scalar(
                    out=bsc[:, r : r + 1],
                    in0=tsc[:, r : r + 1],
                    scalar1=-1.0,
                    scalar2=1.0 / BW,
                    op0=OP.mult,
                    op1=OP.add,
                )
                st_exp.append((r, A_))
                pump()
            while st_exp or st_va or st_tt:
                pump(drain=True)

        # ---------------- phase 3: fold M across partitions -----------------
        with tc.tile_pool(name="tps", bufs=4, space="PSUM") as tps:
            for j in range(P // 128):
                pt = tps.tile([128, 128], bf16, tag="pt")
                nc.tensor.transpose(pt, Macc[:, j * 128 : (j + 1) * 128], ident)
                nc.vector.tensor_reduce(
                    out=mfold[:, j : j + 1], in_=pt, axis=X, op=OP.max
                )
        nc.sync.dma_start(out=m_d[:, :].rearrange("j p -> p j"), in_=mfold)

    _split_excess_waits(nc, mybir, maxw=1)
    return nc


def kernel(x, y):
    from concourse.bass_utils import run_bass_kernel_spmd

    x = np.ascontiguousarray(np.asarray(x, dtype=np.float32))
    y = np.ascontiguousarray(np.asarray(y, dtype=np.float32))
    assert x.shape == (N, C, H, W) and y.shape == (N, C, H, W)

    if "nc" not in _cache:
        _cache["nc"] = _build_nc()
    nc = _cache["nc"]

    in_maps = []
    for c in range(NCORES):
        n, h = c // 2, c % 2
        in_maps.append(
            {
                "xh": np.ascontiguousarray(
                    x[n].reshape(C, P)[:, h * HALF : (h + 1) * HALF]
                ),
                "yn": np.ascontiguousarray(y[n].reshape(C, P)),
            }
        )
    res = run_bass_kernel_spmd(nc, in_maps, core_ids=list(range(NCORES)))
    ms = [r["m_out"].reshape(P) for r in res.results]
    cx = np.empty(N, np.float64)
    for n in range(N):
        m = np.maximum(ms[2 * n], ms[2 * n + 1])
        cx[n] = m.astype(np.float64).mean()
    loss = np.mean(-np.log(cx + EPS))
    return np.asarray(loss, dtype=np.float32)
